# revision 7
# baseline (speedup 1.0000x reference)
"""GraphThreshold (threshold mask -> symmetrize -> coalesce) on 8 Trainium2 cores.

Strategy (hardcoded for edge_index [2, 2_560_000] int32 in [0,40000), edge_attr
[2_560_000] f32, t [1] f32):

Host (sharding): symmetrize edges, then partition by row-range across the 8
devices (device d owns rows [d*5000, (d+1)*5000)).  Within a device, partition p
owns the 40-row block [p*40, (p+1)*40); each (row, col-half) pair is a 128-slot
segment of p's free dim (40 rows x 2 col halves = 80 segments).  Duplicate
(row,col) keys always land in the same segment, so coalesce is segment-local.

Device kernel (Bass/Tile): threshold mask (attr <= t), per-segment sort by col
(Batcher odd-even merge network on the vector engine, all 80*128 segments per
partition processed by each strided compare-exchange op), duplicate coalesce
via segmented multiply-add scan, invalid/dup slots pushed to key >= 65536.

Host (unshard): decode (row from slot position, col, summed attr) for slots
with key < 65536, concatenate in (device, partition, segment, slot) order
(= global (row, col) sorted order), pad to the reference's fixed shapes.
"""

import numpy as np

N_NODES = 40000
E = 2_560_000
E2 = 2 * E
N_CORES = 8
ROWS_PER_DEV = N_NODES // N_CORES  # 5000
ROWS_PER_PART = 40  # 128 parts * 40 = 5120 >= 5000 rows (tail parts empty)
SEGS = 80  # 40 rows x 2 col halves
S = 128  # slots per segment
F = SEGS * S  # 10240 free elems per partition
COL_HALF = N_NODES // 2
SENT = 65536  # > any col; fp32-exact

_CACHE = {}


def _batcher_stages(n):
    p = 1
    while p < n:
        k = p
        while k >= 1:
            yield (p, k)
            k //= 2
        p *= 2


def _build_nc():
    import concourse.bacc as bacc
    import concourse.mybir as mybir
    import concourse.tile as tile

    f32 = mybir.dt.float32
    i32 = mybir.dt.int32
    op = mybir.AluOpType

    nc = bacc.Bacc(None, target_bir_lowering=False)
    col_in = nc.dram_tensor("col", [128, F], f32, kind="ExternalInput")
    attr_in = nc.dram_tensor("attr", [128, F], f32, kind="ExternalInput")
    t_in = nc.dram_tensor("t", [128, 1], f32, kind="ExternalInput")
    okey_out = nc.dram_tensor("okey", [128, F], i32, kind="ExternalOutput")
    oval_out = nc.dram_tensor("oval", [128, F], f32, kind="ExternalOutput")

    with tile.TileContext(nc) as tc:
        with tc.tile_pool(name="sbuf", bufs=1) as pool:
            key = pool.tile([128, F], f32)
            val = pool.tile([128, F], f32)
            tt = pool.tile([128, 1], f32)
            A = pool.tile([128, F], f32)
            B = pool.tile([128, F], f32)

            nc.sync.dma_start(out=key[:], in_=col_in[:])
            nc.sync.dma_start(out=val[:], in_=attr_in[:])
            nc.sync.dma_start(out=tt[:], in_=t_in[:])

            # mask: A = (attr > t); key += A*SENT; val *= (1-A)
            nc.vector.tensor_scalar(
                out=A[:], in0=val[:], scalar1=tt[:], scalar2=None, op0=op.is_gt
            )
            nc.vector.tensor_scalar_mul(out=B[:], in0=A[:], scalar1=float(SENT))
            nc.vector.tensor_add(out=key[:], in0=key[:], in1=B[:])
            nc.vector.tensor_mul(out=B[:], in0=val[:], in1=A[:])
            nc.vector.tensor_tensor(
                out=val[:], in0=val[:], in1=B[:], op=op.subtract
            )

            # per-segment Batcher odd-even merge sort by key, vals follow
            kf, vf = key[:], val[:]

            def views(ap, p, k):
                r = ap.rearrange("q (a r) -> q a r", r=2 * p)
                if k == p:
                    return r[:, :, 0:p], r[:, :, p : 2 * p]
                r2 = r[:, :, k : 2 * p - k]
                r3 = r2.rearrange("q a (b c) -> q a b c", c=2 * k)
                return r3[:, :, :, 0:k], r3[:, :, :, k : 2 * k]

            # scratch (A/B are dead during the sort): flat [128, npair] views
            for p, k in _batcher_stages(S):
                ka, kb = views(kf, p, k)
                va, vb = views(vf, p, k)
                if k == p:
                    npair = (F // (2 * p)) * p
                else:
                    npair = (F // (2 * p)) * (p // k - 1) * k
                c = A[:, :npair]
                t1 = A[:, F // 2 : F // 2 + npair]
                t2 = B[:, :npair]
                # c = (ka > kb) in {0.0, 1.0}; exact value swap via *c
                nc.vector.tensor_tensor(out=c, in0=ka, in1=kb, op=op.is_gt)
                nc.vector.tensor_mul(out=t1, in0=va, in1=c)
                nc.vector.tensor_mul(out=t2, in0=vb, in1=c)
                nc.vector.tensor_tensor(out=va, in0=va, in1=t1, op=op.subtract)
                nc.vector.tensor_tensor(out=vb, in0=vb, in1=t2, op=op.subtract)
                nc.vector.tensor_add(out=va, in0=va, in1=t2)
                nc.vector.tensor_add(out=vb, in0=vb, in1=t1)
                nc.vector.tensor_tensor(out=t1, in0=ka, in1=kb, op=op.min)
                nc.vector.tensor_tensor(out=kb, in0=ka, in1=kb, op=op.max)
                nc.vector.tensor_copy(out=ka, in_=t1)

            # coalesce: A[x] = (key[x]==key[x-1]) within segment ("eq")
            k3 = kf.rearrange("q (s c) -> q s c", c=S)
            a3 = A[:].rearrange("q (s c) -> q s c", c=S)
            v3 = val[:].rearrange("q (s c) -> q s c", c=S)
            nc.vector.memset(A[:], 0.0)
            nc.vector.tensor_tensor(
                out=a3[:, :, 1:], in0=k3[:, :, 1:], in1=k3[:, :, :-1], op=op.is_equal
            )
            # segmented inclusive scan of vals over equal-key runs -> B
            nc.vector.tensor_tensor_scan(
                out=B[:], data0=A[:], data1=val[:], initial=0.0,
                op0=op.mult, op1=op.add,
            )
            # keep (in val) = (key valid) & (next slot not same key)
            nc.vector.memset(val[:], 1.0)
            nc.vector.tensor_scalar(
                out=v3[:, :, :-1], in0=a3[:, :, 1:], scalar1=0.0, scalar2=None,
                op0=op.is_equal,
            )
            nc.vector.tensor_scalar(
                out=A[:], in0=key[:], scalar1=float(SENT), scalar2=None,
                op0=op.is_lt,
            )
            nc.vector.tensor_mul(out=val[:], in0=val[:], in1=A[:])  # keep
            # outputs: oval = sums*keep ; okey = key + (1-keep)*SENT
            nc.vector.tensor_mul(out=B[:], in0=B[:], in1=val[:])
            nc.vector.tensor_scalar(
                out=A[:], in0=val[:], scalar1=0.0, scalar2=None, op0=op.is_equal
            )
            nc.vector.tensor_scalar_mul(out=A[:], in0=A[:], scalar1=float(SENT))
            nc.vector.tensor_add(out=key[:], in0=key[:], in1=A[:])

            oki = pool.tile([128, F], i32)
            nc.vector.tensor_copy(out=oki[:], in_=key[:])
            nc.sync.dma_start(out=okey_out[:], in_=oki[:])
            nc.sync.dma_start(out=oval_out[:], in_=B[:])

    nc.compile()
    return nc


def _get_nc():
    if "nc" not in _CACHE:
        _CACHE["nc"] = _build_nc()
    return _CACHE["nc"]


def kernel(edge_index, edge_attr, t):
    from concourse.bass_utils import run_bass_kernel_spmd

    edge_index = np.asarray(edge_index)
    edge_attr = np.asarray(edge_attr, dtype=np.float32)
    t = np.asarray(t, dtype=np.float32)

    # symmetrize
    r2 = np.concatenate([edge_index[0], edge_index[1]]).astype(np.int64)
    c2 = np.concatenate([edge_index[1], edge_index[0]]).astype(np.int64)
    a2 = np.concatenate([edge_attr, edge_attr])

    # bucket = (device, partition, segment)
    dev = r2 // ROWS_PER_DEV
    rl = r2 % ROWS_PER_DEV
    part = rl // ROWS_PER_PART
    seg = (rl % ROWS_PER_PART) * 2 + (c2 >= COL_HALF)
    bucket = (dev * 128 + part) * SEGS + seg
    n_buckets = N_CORES * 128 * SEGS

    order = np.argsort(bucket, kind="stable")
    sb = bucket[order]
    counts = np.bincount(bucket, minlength=n_buckets)
    assert counts.max() <= S, f"bucket overflow: {counts.max()}"
    offs = np.zeros(n_buckets + 1, np.int64)
    np.cumsum(counts, out=offs[1:])
    within = np.arange(E2, dtype=np.int64) - offs[sb]

    col_arr = np.zeros((n_buckets, S), np.float32)
    attr_arr = np.full((n_buckets, S), 2.0, np.float32)  # pad attr > t -> masked
    col_arr[sb, within] = c2[order].astype(np.float32)
    attr_arr[sb, within] = a2[order]

    col_arr = col_arr.reshape(N_CORES, 128, F)
    attr_arr = attr_arr.reshape(N_CORES, 128, F)
    t_arr = np.full((128, 1), t[0], np.float32)

    in_maps = [
        {"col": col_arr[d], "attr": attr_arr[d], "t": t_arr} for d in range(N_CORES)
    ]
    res = run_bass_kernel_spmd(
        _get_nc(), in_maps, core_ids=list(range(N_CORES))
    )

    # unshard: decode valid slots in global sorted order
    rows_parts = []
    cols_parts = []
    attr_parts = []
    # row of each slot: d*5000 + p*40 + s//2  (constant per segment)
    row_of_seg = (
        np.arange(128)[:, None] * ROWS_PER_PART + np.arange(SEGS)[None, :] // 2
    )  # [128, SEGS]
    row_of_slot = np.broadcast_to(row_of_seg[:, :, None], (128, SEGS, S))
    for d in range(N_CORES):
        k = res.results[d]["okey"].reshape(128, SEGS, S)
        v = res.results[d]["oval"].reshape(128, SEGS, S)
        valid = k < SENT
        rows_parts.append((d * ROWS_PER_DEV + row_of_slot[valid]).astype(np.int32))
        cols_parts.append(k[valid].astype(np.int32))
        attr_parts.append(v[valid])

    rows = np.concatenate(rows_parts)
    cols = np.concatenate(cols_parts)
    attrs = np.concatenate(attr_parts)
    n = rows.shape[0]

    edge_index_out = np.full((2, E2), -1, np.int32)
    edge_attr_out = np.zeros(E2, np.float32)
    edge_index_out[0, :n] = rows
    edge_index_out[1, :n] = cols
    edge_attr_out[:n] = attrs
    return edge_index_out, edge_attr_out, np.int32(n)
